# revision 17
# baseline (speedup 1.0000x reference)
"""CrossAttention TRN2 kernel: 8-core SPMD, shard = (batch, S1-half).

Per core: q rows [1024, 512] of one batch; full k,v [2048,512] of that batch;
all weights. Two-pass softmax attention, tuned for the TRN2 timeline cost
model (matmul cost = output free-dim columns; engines balanced):

  1. PE-transpose q,k,v (fp32) -> qT/kT/vT chunks; round to fp32r on copies.
  2. Projections (fp32r): qhT[65,1024]/khT[65,2048] per head (row 64 of khT
     = 1.0, row 64 of qhT = -rowmax, so S^T comes out max-subtracted);
     vh[t, 8*65] bf16 with a ones column per head (fused rowsum).
  3. Per head h (maxpass for h+1 interleaved, front-loaded so the aug-row
     flush hides behind the head tail):
     - maxpass: S[s-tile, t] fp32r matmuls into rotating [128,1024] psum
       (shared 3-buffer pool with the S^T tiles); DVE X-reduce (negated) ->
       negm cols; per head TT-min + one PE transpose + one DMA into the
       qhT aug row.
     - S^T pass (K=65): [t-tile, s] psum; ACT exp(scale=1/8) -> P^T bf16.
     - PV flipped: o[s-tile, 65] psum accumulates over t-tiles with
       lhsT = P^T block (stationary), rhs = vh column block (65 cols);
       col 64 = softmax denominator. start=True only for the first group
       per psum bank (start zeroes the whole 2KB bank).
  4. Normalize per-partition (DVE reciprocal + ACT/DVE scale-mul) -> outn
     [s, 512] bf16; PE-transpose (every 2 heads) -> outT [hp, s];
     final projection vs Wo bf16 -> out [1024, 512].
"""
import sys
import functools

sys.path.insert(0, "/opt/trn_rl_repo")
import numpy as np
from contextlib import ExitStack

B, S1, S2, D, H, P = 4, 2048, 2048, 512, 8, 64
SC = S1 // 2          # 1024 q rows per core
NCORES = 8
DCH = D // 128        # 4 d-chunks
QT = SC // 128        # 8 q s-tiles
TT = S2 // 128        # 16 t-tiles
TBLK = S2 // 512      # 4 t blocks of 512
SBL = SC // 512       # 2 s blocks of 512
PW = P + 1            # 65: head cols in vh (64 v + ones)


@functools.lru_cache(maxsize=1)
def _build():
    from concourse import bacc, tile, mybir, masks

    f32 = mybir.dt.float32
    f32r = mybir.dt.float32r
    bf16 = mybir.dt.bfloat16
    AX = mybir.AxisListType
    OP = mybir.AluOpType

    nc = bacc.Bacc("TRN2", target_bir_lowering=False, debug=False)

    q_d = nc.dram_tensor("q", [SC, D], f32, kind="ExternalInput").ap()
    k_d = nc.dram_tensor("k", [S2, D], f32, kind="ExternalInput").ap()
    v_d = nc.dram_tensor("v", [S2, D], f32, kind="ExternalInput").ap()
    wq_d = nc.dram_tensor("Wq", [H, D, P], f32, kind="ExternalInput").ap()
    wk_d = nc.dram_tensor("Wk", [H, D, P], f32, kind="ExternalInput").ap()
    wv_d = nc.dram_tensor("Wv", [H, D, P], f32, kind="ExternalInput").ap()
    wo_d = nc.dram_tensor("Wo", [H * P, D], f32, kind="ExternalInput").ap()
    out_d = nc.dram_tensor("out", [SC, D], f32, kind="ExternalOutput").ap()

    with tile.TileContext(nc) as tc, ExitStack() as ctx:
        const_pool = ctx.enter_context(tc.tile_pool(name="const", bufs=1))
        ident = const_pool.tile([128, 128], f32)
        masks.make_identity(nc, ident[:])
        ident_bf = const_pool.tile([128, 128], bf16)
        masks.make_identity(nc, ident_bf[:])

        # ---- persistent tiles ----
        wpool = ctx.enter_context(tc.tile_pool(name="wr", bufs=1))
        wq_r = [wpool.tile([128, H * P], f32r, tag=f"wq{c}", name=f"wq{c}") for c in range(DCH)]
        wk_r = [wpool.tile([128, H * P], f32r, tag=f"wk{c}", name=f"wk{c}") for c in range(DCH)]
        wv_r = [wpool.tile([128, H * P], f32r, tag=f"wv{c}", name=f"wv{c}") for c in range(DCH)]
        wo_bf = [wpool.tile([128, D], bf16, tag=f"wo{c}", name=f"wo{c}") for c in range(DCH)]

        nat_pool = ctx.enter_context(tc.tile_pool(name="nat", bufs=8))
        act_pool = ctx.enter_context(tc.tile_pool(name="acts", bufs=1))
        qhT = [act_pool.tile([65, SC], f32r, tag=f"qhT{h}", name=f"qhT{h}") for h in range(H)]
        khT = [act_pool.tile([65, S2], f32r, tag=f"khT{h}", name=f"khT{h}") for h in range(H)]
        # vh: per t-tile [128, 8*65] bf16; per head 64 v cols + ones col
        vh = [act_pool.tile([128, H * PW], bf16, tag=f"vh{t}", name=f"vh{t}") for t in range(TT)]

        ones_row = const_pool.tile([1, S2], f32)
        nc.vector.memset(ones_row[:], 1.0)
        for h in range(H):
            nc.scalar.copy(khT[h][64:65, :], ones_row[:])

        def load_weights(names):
            """Batched 3D-AP weight DMAs; round to fp32r / cast bf16 on Pool."""
            with tc.tile_pool(name="wtmp", bufs=1) as wtmp_pool:
                for wname, name_d, dst in (("wq", wq_d, wq_r), ("wk", wk_d, wk_r),
                                           ("wv", wv_d, wv_r)):
                    if wname not in names:
                        continue
                    for c in range(DCH):
                        wt = wtmp_pool.tile([128, H * P], f32, tag=f"wt{c}",
                                            name=f"wt_{wname}{c}")
                        nc.sync.dma_start(
                            wt[:].rearrange("d (h p) -> d h p", h=H),
                            name_d[:, c * 128:(c + 1) * 128, :].rearrange("h d p -> d h p"),
                        )
                        nc.gpsimd.tensor_copy(dst[c][:], wt[:])
                if "wo" in names:
                    for c in range(DCH):
                        wt = wtmp_pool.tile([128, D], f32, tag=f"wto{c}", name=f"wto{c}")
                        nc.sync.dma_start(wt[:], wo_d[c * 128:(c + 1) * 128, :])
                        nc.gpsimd.tensor_copy(wo_bf[c][:], wt[:])

        def transpose_round(src_d, nrows, dstT):
            """src_d [nrows, D] fp32 DRAM -> dstT[c] [128, nrows] fp32r."""
            with tc.tile_pool(name="tp_ps", bufs=2, space="PSUM") as tp_ps:
                ntile = nrows // 128
                for g in range(ntile // 4):
                    nats = []
                    for j in range(4):
                        si = 4 * g + j
                        nat = nat_pool.tile([128, D], f32, tag="nat")
                        nc.sync.dma_start(nat[:], src_d[si * 128:(si + 1) * 128, :])
                        nats.append(nat)
                    for c in range(DCH):
                        ps = tp_ps.tile([128, 512], f32)
                        for j in range(4):
                            nc.tensor.transpose(
                                ps[:, j * 128:(j + 1) * 128],
                                nats[j][:, c * 128:(c + 1) * 128],
                                ident[:],
                            )
                        eng = (nc.vector.tensor_copy,
                               nc.scalar.copy)[(g * DCH + c) % 2]
                        eng(dstT[c][:, g * 512:(g + 1) * 512], ps[:])

        small_pool = ctx.enter_context(tc.tile_pool(name="small", bufs=2))
        pt_pool = ctx.enter_context(tc.tile_pool(name="pt", bufs=4))

        def maxpass_half(h, idx, negm2, pool):
            """-rowmax of S[s-tile idx//2, t-half idx%2] for head h."""
            qi, half = idx // 2, idx % 2
            mx = pool.tile([128, 1024], f32, tag="big", name=f"mx{h}_{idx}")
            for tb in range(2):
                nc.tensor.matmul(
                    mx[:, tb * 512:(tb + 1) * 512],
                    qhT[h][0:64, qi * 128:(qi + 1) * 128],
                    khT[h][0:64, (2 * half + tb) * 512:(2 * half + tb + 1) * 512],
                    start=True, stop=True,
                )
            nc.vector.tensor_reduce(
                negm2[:, 2 * qi + half:2 * qi + half + 1], mx[:],
                axis=AX.X, op=OP.max, negate=True,
            )

        def flush_negm(h, negm2, pool):
            """Combine half-maxes, transpose, DMA into qhT[h] aug row."""
            negm_all = small_pool.tile([128, QT], f32, tag="negma", name=f"negma{h}")
            nc.vector.tensor_tensor(
                negm_all[:], negm2[:, 0:2 * QT:2], negm2[:, 1:2 * QT:2], op=OP.min,
            )
            tpt = pool.tile([128, 1024], f32, tag="big", name=f"ntp{h}")
            tp = tpt[0:QT, 0:128]
            nc.tensor.transpose(tp, negm_all[:], ident[:])
            nrow = small_pool.tile([QT, 128], f32r, tag="nrow", name=f"nrow{h}")
            nc.vector.tensor_copy(nrow[:], tp)
            nc.sync.dma_start(
                qhT[h][64:65, :].rearrange("o (g c) -> o g c", g=QT),
                nrow[:, :],
            )

        negm_tiles = {}

        # ---- q path (q DMAs first so PE starts ASAP; weights trail) ----
        with tc.tile_pool(name="qT", bufs=1) as qT_pool, \
             tc.tile_pool(name="proj_ps", bufs=3, space="PSUM") as proj_ps:
            qT = [qT_pool.tile([128, SC], f32r, tag=f"qT{c}", name=f"qT{c}") for c in range(DCH)]
            load_weights(("wq",))
            transpose_round(q_d, SC, qT)
            load_weights(("wk", "wv", "wo"))
            for hp in range(H // 2):
                for sb in range(SBL):
                    ps = proj_ps.tile([128, 512], f32)
                    for c in range(DCH):
                        nc.tensor.matmul(
                            ps[:],
                            wq_r[c][:, hp * 128:(hp + 1) * 128],
                            qT[c][:, sb * 512:(sb + 1) * 512],
                            start=(c == 0), stop=(c == DCH - 1),
                        )
                    eng = nc.scalar.copy if sb == 0 else nc.vector.tensor_copy
                    eng(qhT[2 * hp][0:64, sb * 512:(sb + 1) * 512], ps[0:64, :])
                    eng(qhT[2 * hp + 1][0:64, sb * 512:(sb + 1) * 512], ps[64:128, :])

        # ---- k path ----
        with tc.tile_pool(name="kT", bufs=1) as kT_pool, \
             tc.tile_pool(name="proj_ps2", bufs=3, space="PSUM") as proj_ps:
            kT = [kT_pool.tile([128, S2], f32r, tag=f"kT{c}", name=f"kT{c}") for c in range(DCH)]
            transpose_round(k_d, S2, kT)
            negm_tiles[0] = small_pool.tile([128, 2 * QT], f32, tag="negm", name="negm0")
            with tc.tile_pool(name="mxpro", bufs=1, space="PSUM") as mxpro:
                mh0 = 0
                for hp in range(H // 2):
                    for tb in range(TBLK):
                        ps = proj_ps.tile([128, 512], f32)
                        for c in range(DCH):
                            nc.tensor.matmul(
                                ps[:],
                                wk_r[c][:, hp * 128:(hp + 1) * 128],
                                kT[c][:, tb * 512:(tb + 1) * 512],
                                start=(c == 0), stop=(c == DCH - 1),
                            )
                        eng = nc.scalar.copy if tb % 2 == 0 else nc.vector.tensor_copy
                        eng(khT[2 * hp][0:64, tb * 512:(tb + 1) * 512], ps[0:64, :])
                        eng(khT[2 * hp + 1][0:64, tb * 512:(tb + 1) * 512], ps[64:128, :])
                        if hp >= 1:
                            # head-0 maxpass interleaved (khT[0]/qhT[0] ready)
                            tgt = min(2 * QT, ((hp - 1) * TBLK + tb + 1) * 2)
                            while mh0 < tgt:
                                maxpass_half(0, mh0, negm_tiles[0], mxpro)
                                mh0 += 1
                while mh0 < 2 * QT:
                    maxpass_half(0, mh0, negm_tiles[0], mxpro)
                    mh0 += 1
                flush_negm(0, negm_tiles.pop(0), mxpro)

        # ---- v path (strided copy into 65-col-per-head layout + ones col) ----
        with tc.tile_pool(name="vT", bufs=1) as vT_pool, \
             tc.tile_pool(name="proj_ps3", bufs=3, space="PSUM") as proj_ps:
            vT = [vT_pool.tile([128, S2], f32r, tag=f"vT{c}", name=f"vT{c}") for c in range(DCH)]
            transpose_round(v_d, S2, vT)
            for ti in range(TT):
                ps = proj_ps.tile([128, 512], f32)
                for c in range(DCH):
                    nc.tensor.matmul(
                        ps[:],
                        vT[c][:, ti * 128:(ti + 1) * 128],
                        wv_r[c][:],
                        start=(c == 0), stop=(c == DCH - 1),
                    )
                vdst = vh[ti][:].rearrange("t (h c) -> t h c", h=H)
                eng = nc.vector.tensor_copy if ti % 2 == 0 else nc.scalar.copy
                eng(vdst[:, :, 0:P], ps[:].rearrange("t (h p) -> t h p", h=H))
                nc.vector.memset(vdst[:, :, P:PW], 1.0)

        # ---- attention ----
        fin_pool = ctx.enter_context(tc.tile_pool(name="fin", bufs=1))
        outn = [fin_pool.tile([128, H * P], bf16, tag=f"onorm{sc}", name=f"onorm{sc}") for sc in range(QT)]
        outT = [fin_pool.tile([128, SC], bf16, tag=f"outT{c}", name=f"outT{c}") for c in range(DCH)]

        with tc.tile_pool(name="big_ps", bufs=3, space="PSUM") as big_ps, \
             tc.tile_pool(name="oacc_ps", bufs=1, space="PSUM") as oacc_ps:

            st_tiles = {}

            def emit_st(h, ti):
                if (h, ti) in st_tiles:
                    return
                st = big_ps.tile([128, 1024], f32, tag="big", name=f"st{h}_{ti}")
                for sb in range(SBL):
                    nc.tensor.matmul(
                        st[:, sb * 512:(sb + 1) * 512],
                        khT[h][0:65, ti * 128:(ti + 1) * 128],
                        qhT[h][0:65, sb * 512:(sb + 1) * 512],
                        start=True, stop=True,
                    )
                st_tiles[(h, ti)] = st

            otp_todo = []

            def emit_otp(c, sc):
                tpt2 = big_ps.tile([128, 1024], f32, tag="big", name=f"otp{c}_{sc}")
                tps2 = tpt2[:, 0:64].bitcast(bf16)
                nc.tensor.transpose(
                    tps2, outn[sc][:, c * 128:(c + 1) * 128], ident_bf[:],
                )
                nc.vector.tensor_copy(outT[c][:, sc * 128:(sc + 1) * 128], tps2)

            for h in range(H):
                if h + 1 < H:
                    negm_tiles[h + 1] = small_pool.tile([128, 2 * QT], f32, tag="negm",
                                                        name=f"negm{h + 1}")
                # two [128, 4*65] accumulators (each within one psum bank)
                oacc = [oacc_ps.tile([128, 4 * PW], f32, tag=f"oacc{i}", name=f"oacc{h}_{i}")
                        for i in range(2)]

                emit_st(h, 0)
                mh = 0  # maxpass halves emitted for h+1
                for ti in range(TT):
                    if ti + 1 < TT:
                        emit_st(h, ti + 1)
                    if h + 1 < H:
                        # front-load: all 16 halves done by ti=13, flush at 14
                        tgt = min(2 * QT, ti + 3)
                        while mh < tgt:
                            maxpass_half(h + 1, mh, negm_tiles[h + 1], big_ps)
                            mh += 1
                        if mh == 2 * QT and ti == TT - 2:
                            flush_negm(h + 1, negm_tiles.pop(h + 1), big_ps)
                            mh += 1
                    if h == H - 1:
                        # drain deferred outn->outT transposes in the idle last head
                        while otp_todo and len(otp_todo) > 24 - (ti + 1) * 24 // TT:
                            emit_otp(*otp_todo.pop(0))
                    ptile = pt_pool.tile([128, 1024], bf16, tag="pt", name=f"pt{h}_{ti}")
                    nc.scalar.activation(ptile[:], st_tiles.pop((h, ti))[:],
                                         mybir.ActivationFunctionType.Exp, scale=0.125)
                    for sc in range(QT):
                        # start=True zeroes the whole 2KB psum bank; only the
                        # first group per bank may set it.
                        nc.tensor.matmul(
                            oacc[sc // 4][:, (sc % 4) * PW:(sc % 4 + 1) * PW],
                            ptile[:, sc * 128:(sc + 1) * 128],
                            vh[ti][:, h * PW:(h + 1) * PW],
                            start=(ti == 0 and sc % 4 == 0), stop=(ti == TT - 1),
                            skip_group_check=(sc % 4 != 0),
                        )
                # pre-emit next head's first S^T tiles to cover the tail
                if h + 1 < H:
                    emit_st(h + 1, 0)
                    emit_st(h + 1, 1)
                    emit_st(h + 1, 2)

                # normalize: outn[sc][:, h*64:(h+1)*64] = o * (1/rowsum);
                # all recips first (one [128,8] tile) to shorten the oacc
                # WAR chain gating the next head's PV start
                rec8 = small_pool.tile([128, QT], f32, tag="rec", name=f"rec{h}")
                for sc in range(QT):
                    acc = oacc[sc // 4]
                    base = (sc % 4) * PW
                    nc.vector.reciprocal(rec8[:, sc:sc + 1], acc[:, base + P:base + PW])
                for sc in range(QT):
                    acc = oacc[sc // 4]
                    base = (sc % 4) * PW
                    eng = nc.scalar.mul if sc % 2 == 0 else nc.vector.tensor_scalar_mul
                    eng(
                        outn[sc][:, h * P:(h + 1) * P], acc[:, base:base + P],
                        rec8[:, sc:sc + 1],
                    )
                if h % 2 == 1:
                    c = h // 2
                    if h < H - 1:
                        otp_todo.extend((c, sc) for sc in range(QT))
                    else:
                        for item in otp_todo:
                            emit_otp(*item)
                        for sc in range(QT):
                            emit_otp(c, sc)

        # ---- final projection ----
        with tc.tile_pool(name="fin_ps", bufs=2, space="PSUM") as fin_ps, \
             tc.tile_pool(name="fin_sb", bufs=2) as fin_sb_pool:
            for sc in range(QT):
                ps = fin_ps.tile([128, 512], f32, tag="fps", name=f"fps{sc}")
                for c in range(DCH):
                    nc.tensor.matmul(
                        ps[:],
                        outT[c][:, sc * 128:(sc + 1) * 128],
                        wo_bf[c][:],
                        start=(c == 0), stop=(c == DCH - 1),
                    )
                fin = fin_sb_pool.tile([128, 512], f32, tag="fin", name=f"fin{sc}")
                eng = nc.vector.tensor_copy if sc % 2 == 0 else nc.scalar.copy
                eng(fin[:], ps[:])
                nc.sync.dma_start(out_d[sc * 128:(sc + 1) * 128, :], fin[:])

    nc.compile()
    return nc


def kernel(q, k, v, Wq, Wk, Wv, Wo):
    nc = _build()
    from concourse.bass_utils import run_bass_kernel_spmd

    q = np.asarray(q, np.float32)
    k = np.asarray(k, np.float32)
    v = np.asarray(v, np.float32)
    in_maps = []
    for c in range(NCORES):
        b, half = c // 2, c % 2
        in_maps.append({
            "q": np.ascontiguousarray(q[b, half * SC:(half + 1) * SC, :]),
            "k": np.ascontiguousarray(k[b]),
            "v": np.ascontiguousarray(v[b]),
            "Wq": np.ascontiguousarray(Wq, dtype=np.float32),
            "Wk": np.ascontiguousarray(Wk, dtype=np.float32),
            "Wv": np.ascontiguousarray(Wv, dtype=np.float32),
            "Wo": np.ascontiguousarray(Wo, dtype=np.float32),
        })
    res = run_bass_kernel_spmd(nc, in_maps, core_ids=list(range(NCORES)))
    globals()["LAST_RES"] = res
    out = np.empty((B, S1, D), np.float32)
    for c, r in enumerate(res.results):
        b, half = c // 2, c % 2
        out[b, half * SC:(half + 1) * SC] = r["out"]
    return out


if __name__ == "__main__":
    rng = np.random.default_rng(0)
    qq = rng.standard_normal((B, S1, D), dtype=np.float32)
    kk = rng.standard_normal((B, S2, D), dtype=np.float32)
    vv = rng.standard_normal((B, S2, D), dtype=np.float32)
    wq = rng.standard_normal((H, D, P), dtype=np.float32)
    wk = rng.standard_normal((H, D, P), dtype=np.float32)
    wv = rng.standard_normal((H, D, P), dtype=np.float32)
    wo = rng.standard_normal((H * P, D), dtype=np.float32)
    o = kernel(qq, kk, vv, wq, wk, wv, wo)
    print("out", o.shape, o.dtype, np.abs(o).mean())
